# revision 36
# baseline (speedup 1.0000x reference)
"""Local (causal, windowed) attention block on 8 trn2 NeuronCores.

Sharding: sequence-parallel. 8 shards = batch(2) x seq-quarter(4); each core
computes 512 output tokens and needs a 256-token K/V halo on the left.

All matmul operands are bf16 (psum accumulation stays f32): same 1 cycle/row
PE rate as fp32r but half the DMA/SBUF traffic and less PE power (fp32r at
full rate trips the DVFS utilization throttle). Data flows in transposed
[feature, token] layout so Q/K/scores/attn-out chain without transposes.
Softmax denominators come from an appended ones column in the V operand;
masking is multiplicative bf16 applied post-exp in SBUF (band masks doubled
per head pair), split across Pool and DVE. bv and bo fold host-side into
bo_eff = bv@wo + bo (softmax rows sum to 1), added via a ones-row matmul in
phase E. The attention loop is software-pipelined by one head pair; each
(pair, chunk) gets a two-bank psum tile (one accumulation group per bank --
sharing a bank hangs the HW) so a single strided exp covers both heads.
HW quirks found on the way: custom-DVE reciprocal must read SBUF at base
partition 0 (psum or base-64 reads give garbage), SB+SB tensor_tensor
operands must share a base partition, and stale SBUF from prior NEFF runs
must never reach a matmul (0 x garbage-NaN = NaN).
"""

import numpy as np
import ml_dtypes

import concourse.bass as bass  # noqa: F401
import concourse.mybir as mybir
import concourse.tile as tile
from concourse import bacc
from concourse.bass_utils import run_bass_kernel_spmd

B, S, D = 2, 2048, 1024
H, DH = 16, 64
WIN = 256
TOK, HALO = 512, 256
XT = TOK + HALO  # 768
F32 = mybir.dt.float32
F32R = mybir.dt.float32r
BF16 = mybir.dt.bfloat16

# per-key-chunk query windows (qlo, qwidth); kc=3 widened to 512 so the
# first attn@V matmul (start=True) covers the whole psum bank.
KC_WIN = [(0, 128), (0, 256), (0, 384), (0, 512), (256, 256), (384, 128)]
KC_ORDER = [3, 0, 1, 2, 4, 5]

_cache = {}


def r(ap):
    return ap.bitcast(F32R)


def build_nc():
    nc = bacc.Bacc(None, target_bir_lowering=False)
    xh_d = nc.declare_dram_parameter("xh", [XT, D], BF16, isOutput=False)
    msk2_d = nc.declare_dram_parameter("msk2", [6, 128, 1024], BF16, isOutput=False)
    wq_d = nc.declare_dram_parameter("wq", [D, D], BF16, isOutput=False)
    wk_d = nc.declare_dram_parameter("wk", [D, D], BF16, isOutput=False)
    wv_d = nc.declare_dram_parameter("wv", [D, D], BF16, isOutput=False)
    wo_d = nc.declare_dram_parameter("wo", [D, D], BF16, isOutput=False)
    bq_d = nc.declare_dram_parameter("bq", [D], F32, isOutput=False)  # pre-scaled 1/8
    bk_d = nc.declare_dram_parameter("bk", [D], F32, isOutput=False)
    ones16_d = nc.declare_dram_parameter("ones16", [128, 16], BF16, isOutput=False)
    onesr_d = nc.declare_dram_parameter("onesr", [1, 128], BF16, isOutput=False)
    bob16_d = nc.declare_dram_parameter("bob16", [1, D], BF16, isOutput=False)
    ident_d = nc.declare_dram_parameter("ident", [128, 128], BF16, isOutput=False)
    out_d = nc.declare_dram_parameter("out", [TOK, D], F32, isOutput=True)

    Exp = mybir.ActivationFunctionType.Exp
    Ident = mybir.ActivationFunctionType.Identity

    with tile.TileContext(nc) as tc:
        with (
            tc.tile_pool(name="const", bufs=1) as const,
            tc.tile_pool(name="persist", bufs=1) as persist,
            tc.tile_pool(name="w", bufs=9) as wpool,
        ):

            QT = [persist.tile([128, TOK], BF16, name=f"QT{i}") for i in range(8)]
            KT = [persist.tile([128, XT], BF16, name=f"KT{i}") for i in range(8)]
            Vg = [persist.tile([128, 16 * 65], BF16, name=f"Vg{i}") for i in range(6)]
            AO = [persist.tile([128, TOK], BF16, name=f"AO{i}") for i in range(8)]

            # ---- Phase A: load x (bf16), transpose to xT [feat, tok] ----
            ident = const.tile([128, 128], BF16)
            nc.sync.dma_start(out=ident[:], in_=ident_d[:, :])
            with tc.tile_pool(name="xTp", bufs=1) as xTp:
                xTt = [xTp.tile([128, XT], BF16, name=f"xT{i}") for i in range(8)]
                with (
                    tc.tile_pool(name="xload", bufs=3) as xpool,
                    tc.tile_pool(name="tpsum", bufs=4, space="PSUM") as tpsum,
                ):
                    for tt in range(6):
                        xt = xpool.tile([128, D], BF16, tag="xt")
                        nc.sync.dma_start(out=xt[:], in_=xh_d[tt * 128:(tt + 1) * 128, :])
                        for fc in range(8):
                            pt = tpsum.tile([128, 128], BF16, tag="tp")
                            nc.tensor.transpose(pt[:], xt[:, fc * 128:(fc + 1) * 128], ident[:])
                            if (tt * 8 + fc) % 2 == 0:
                                nc.vector.tensor_copy(xTt[fc][:, tt * 128:(tt + 1) * 128], pt[:])
                            else:
                                nc.scalar.copy(xTt[fc][:, tt * 128:(tt + 1) * 128], pt[:])
                wq_sb0 = []
                for kc in range(8):
                    wt = wpool.tile([128, D], BF16, tag="w", name=f"wq{kc}")
                    nc.sync.dma_start(out=wt[:], in_=wq_d[kc * 128:(kc + 1) * 128, :])
                    wq_sb0.append(wt)

                # ---- Phase B: QT = (wq.T @ xT)/8 + bq/8 ; KT = wk.T @ xT + bk ----
                wq_sb = wq_sb0
                # per-outcol-chunk bias columns: [:, 0:8]=bq/8, [:, 8:16]=bk
                bqk = const.tile([128, 16], F32)
                nc.sync.dma_start(out=bqk[:, 0:8], in_=bq_d[:].rearrange("(c p) -> p c", p=128))
                nc.sync.dma_start(out=bqk[:, 8:16], in_=bk_d[:].rearrange("(c p) -> p c", p=128))
                wk_sb = []
                for kc in range(8):
                    wt = wpool.tile([128, D], BF16, tag="w", name=f"wk{kc}")
                    nc.sync.dma_start(out=wt[:], in_=wk_d[kc * 128:(kc + 1) * 128, :])
                    wk_sb.append(wt)
                m2_sb = []
                for kc in range(6):
                    mt = const.tile([128, 1024], BF16, name=f"mask2_{kc}")
                    nc.sync.dma_start(out=mt[:], in_=msk2_d[kc])
                    m2_sb.append(mt)

                with tc.tile_pool(name="qpsum", bufs=4, space="PSUM") as qpsum:
                    for oc in range(8):
                        ps = qpsum.tile([128, TOK], F32, tag="pp")
                        for kc in range(8):
                            nc.tensor.matmul(
                                ps[:],
                                lhsT=wq_sb[kc][:, oc * 128:(oc + 1) * 128],
                                rhs=xTt[kc][:, HALO:XT],
                                start=(kc == 0), stop=(kc == 7),
                            )
                        # QT pre-scaled by 1/8 (bias arrives pre-scaled from host)
                        nc.scalar.activation(QT[oc][:], ps[:], Ident,
                                             bias=bqk[:, oc:oc + 1], scale=0.125)
                    for oc in range(8):
                        for hf in range(2):
                            ps = qpsum.tile([128, 384], F32, tag="pp", padded_shape=[128, 512])
                            for kc in range(8):
                                nc.tensor.matmul(
                                    ps[:],
                                    lhsT=wk_sb[kc][:, oc * 128:(oc + 1) * 128],
                                    rhs=xTt[kc][:, hf * 384:(hf + 1) * 384],
                                    start=(kc == 0), stop=(kc == 7),
                                )
                            nc.scalar.activation(KT[oc][:, hf * 384:(hf + 1) * 384],
                                                 ps[:], Ident,
                                                 bias=bqk[:, 8 + oc:9 + oc], scale=1.0)

                    # ---- Phase C: V (natural layout, no bias) + ones column ----
                    wv_sb = []
                    for kc in range(8):
                        wt = wpool.tile([128, D], BF16, tag="w", name=f"wv{kc}")
                        nc.sync.dma_start(out=wt[:], in_=wv_d[kc * 128:(kc + 1) * 128, :])
                        wv_sb.append(wt)
                    for tt in range(6):
                        for hf in range(2):
                            ps = qpsum.tile([128, 512], F32, tag="pp")
                            for kc in range(8):
                                nc.tensor.matmul(
                                    ps[:],
                                    lhsT=xTt[kc][:, tt * 128:(tt + 1) * 128],
                                    rhs=wv_sb[kc][:, hf * 512:(hf + 1) * 512],
                                    start=(kc == 0), stop=(kc == 7),
                                )
                            dst = Vg[tt][:, hf * 520:(hf + 1) * 520].rearrange(
                                "p (h d) -> p h d", d=65)[:, :, 0:64]
                            nc.scalar.copy(dst, ps[:].rearrange("p (h d) -> p h d", d=64))
                        nc.sync.dma_start(
                            out=Vg[tt][:].rearrange("p (h d) -> p h d", d=65)[:, :, 64:65],
                            in_=ones16_d[:, 0:16])

            # ---- Phase D: attention (head loop software-pipelined by 1) ----
            # prefetch wo while attention runs
            wo_sb = []
            for kc in range(8):
                wt = wpool.tile([128, D], BF16, tag="w", name=f"wo{kc}")
                nc.sync.dma_start(out=wt[:], in_=wo_d[kc * 128:(kc + 1) * 128, :])
                wo_sb.append(wt)
            onesr = wpool.tile([1, 128], BF16, name="onesr")
            nc.sync.dma_start(out=onesr[:], in_=onesr_d[:, :])
            bob16 = wpool.tile([1, D], BF16, name="bob16")
            nc.sync.dma_start(out=bob16[:], in_=bob16_d[:, :])

            with (
                tc.tile_pool(name="spsum", bufs=2, space="PSUM") as spsum,
                tc.tile_pool(name="opsum", bufs=2, space="PSUM") as opsum,
                tc.tile_pool(name="bpsum", bufs=2, space="PSUM") as bpsum,
                tc.tile_pool(name="es", bufs=17) as es_pool,
                tc.tile_pool(name="aou", bufs=3) as aou_pool,
                tc.tile_pool(name="den", bufs=2) as den_pool,
            ):
                es_all = [None] * 8

                def emit_scores_pair(p):
                    h0, h1 = 2 * p, 2 * p + 1
                    g = p
                    es_slices = {}
                    for kc in KC_ORDER:
                        qlo, qw = KC_WIN[kc]
                        # one two-bank psum tile per (pair, kc): each head's
                        # matmul owns a full bank (one accumulation group per
                        # bank — sharing a bank hangs the HW); a single exp
                        # reads both banks through a strided AP
                        ps = spsum.tile([128, 1024], F32, tag="sp")
                        for h2 in (0, 1):
                            ho = h2 * 64
                            nc.tensor.matmul(
                                ps[:, 512 * h2:512 * h2 + qw],
                                lhsT=KT[g][ho:ho + 64, kc * 128:(kc + 1) * 128],
                                rhs=QT[g][ho:ho + 64, qlo:qlo + qw],
                                start=True, stop=True,
                            )
                        es = es_pool.tile([128, 2 * qw], BF16, tag="es",
                                          padded_shape=[128, 1024])
                        nc.scalar.activation(
                            es[:].rearrange("p (b q) -> p b q", b=2),
                            ps[:].rearrange("p (b q) -> p b q", b=2)[:, :, 0:qw],
                            Exp)
                        with nc.allow_low_precision(reason="bf16 es mask"):
                            eng = nc.gpsimd if kc in (0, 1, 4, 5) else nc.vector
                            eng.tensor_mul(es[:], es[:], m2_sb[kc][:, 0:2 * qw])
                        es_slices[(0, kc)] = es[:, 0:qw]
                        es_slices[(1, kc)] = es[:, qw:2 * qw]
                    es_all[p] = es_slices

                def emit_av_pair(p):
                    g = p
                    es_slices = es_all[p]
                    aop = aou_pool.tile([128, TOK], F32, tag="ao")
                    den2 = den_pool.tile([1, 2 * TOK], F32, tag="dn")
                    for h2 in (0, 1):
                        h = 2 * p + h2
                        po = opsum.tile([65, TOK], F32, tag="op")
                        for i, kc in enumerate(KC_ORDER):
                            qlo, qw = KC_WIN[kc]
                            nc.tensor.matmul(
                                po[:, qlo:qlo + qw],
                                lhsT=Vg[kc][:, h * 65:(h + 1) * 65],
                                rhs=es_slices[(h2, kc)],
                                start=(i == 0), stop=(i == 5),
                            )
                        # evacuate psum immediately so the bank recycles fast;
                        # denominators pack into one base-0 row (the custom DVE
                        # reciprocal misreads non-zero base partitions on HW)
                        nc.vector.tensor_copy(aop[64 * h2:64 * h2 + 64, :], po[0:64, :])
                        nc.vector.tensor_copy(den2[:, h2 * TOK:(h2 + 1) * TOK],
                                               po[64:65, :])
                    rsc2 = den_pool.tile([1, 2 * TOK], F32, tag="rs")
                    nc.vector.reciprocal_approx_fast(rsc2[:], den2[:])
                    rcb2 = den_pool.tile([1, 2 * TOK], BF16, tag="rb")
                    with nc.allow_low_precision(reason="bf16 1/den"):
                        nc.scalar.copy(rcb2[:], rsc2[:])
                    pb0 = bpsum.tile([64, TOK], F32, tag="bp")
                    nc.tensor.matmul(pb0[:], lhsT=onesr[:, 0:64],
                                     rhs=rcb2[:, 0:TOK], start=True, stop=True)
                    pb1 = bpsum.tile([64, TOK], F32, tag="bp")
                    nc.tensor.matmul(pb1[:], lhsT=onesr[:, 0:64],
                                     rhs=rcb2[:, TOK:2 * TOK], start=True, stop=True)
                    with nc.allow_low_precision(reason="bf16 attn output"):
                        nc.vector.tensor_mul(AO[g][0:64, :], pb0[:], aop[0:64, :])
                        nc.vector.tensor_mul(AO[g][64:128, :], pb1[:], aop[64:128, :])

                for p in range(9):
                    if p < 8:
                        emit_scores_pair(p)
                    if p >= 1:
                        emit_av_pair(p - 1)

            # ---- Phase E: out = AO.T @ wo + bo_eff ----
            with (
                tc.tile_pool(name="fpsum", bufs=5, space="PSUM") as fpsum,
                tc.tile_pool(name="oout", bufs=3) as oout,
            ):
                for tt in range(4):
                    ot = oout.tile([128, D], F32, tag="oo")
                    for hf in range(2):
                        ps = fpsum.tile([128, 512], F32, tag="fp")
                        nc.tensor.matmul(
                            ps[:], lhsT=onesr[:],
                            rhs=bob16[:, hf * 512:(hf + 1) * 512],
                            start=True, stop=False,
                        )
                        for kc in range(8):
                            nc.tensor.matmul(
                                ps[:],
                                lhsT=AO[kc][:, tt * 128:(tt + 1) * 128],
                                rhs=wo_sb[kc][:, hf * 512:(hf + 1) * 512],
                                start=False, stop=(kc == 7),
                            )
                        nc.scalar.copy(ot[:, hf * 512:(hf + 1) * 512], ps[:])
                        nc.sync.dma_start(
                            out=out_d[tt * 128:(tt + 1) * 128, hf * 512:(hf + 1) * 512],
                            in_=ot[:, hf * 512:(hf + 1) * 512])

    nc.compile()
    return nc


def _mask_for_chunk(c):
    m = np.zeros((6, 128, 512), np.float32)
    for kc in range(6):
        k = kc * 128 + np.arange(128)[:, None]
        q = np.arange(512)[None, :]
        valid = (q >= k - WIN) & (q <= k)
        if c == 0:
            valid = valid & (k >= HALO)
        m[kc][valid] = 1.0
    return m.astype(ml_dtypes.bfloat16)


def kernel(x, wq, bq, wk, bk, wv, bv, wo, bo):
    bf = ml_dtypes.bfloat16
    x = np.asarray(x, np.float32)
    wq16 = np.ascontiguousarray(np.asarray(wq, np.float32).astype(bf))
    wk16 = np.ascontiguousarray(np.asarray(wk, np.float32).astype(bf))
    wv16 = np.ascontiguousarray(np.asarray(wv, np.float32).astype(bf))
    wo32 = np.asarray(wo, np.float32)
    wo16 = np.ascontiguousarray(wo32.astype(bf))
    bq8 = np.ascontiguousarray(np.asarray(bq, np.float32) * 0.125)
    bk = np.ascontiguousarray(np.asarray(bk, np.float32))
    # fold bv through wo (softmax rows sum to 1): out += bv @ wo + bo
    bo_eff = (np.asarray(bv, np.float32) @ wo32 + np.asarray(bo, np.float32)).astype(np.float32)
    bob16 = np.ascontiguousarray(bo_eff.reshape(1, D).astype(bf))
    onesr = np.ones((1, 128), bf)
    ident = np.eye(128, dtype=np.float32).astype(bf)
    ones16 = np.ones((128, 16), bf)

    if "nc" not in _cache:
        _cache["nc"] = build_nc()
        _cache["masks"] = [_mask_for_chunk(c) for c in range(4)]
        m2s = []
        for c in range(4):
            m = _cache["masks"][c]
            m2 = np.zeros((6, 128, 1024), np.float32).astype(m.dtype)
            for kc in range(6):
                qlo, qw = KC_WIN[kc]
                sl = m[kc][:, qlo:qlo + qw]
                m2[kc][:, 0:qw] = sl
                m2[kc][:, qw:2 * qw] = sl
            m2s.append(m2)
        _cache["m2s"] = m2s
    nc = _cache["nc"]
    masks = _cache["masks"]
    m2s = _cache["m2s"]

    in_maps = []
    for core in range(8):
        b, c = divmod(core, 4)
        start = c * TOK
        xh = np.zeros((XT, D), np.float32)
        lo = max(0, start - HALO)
        xh[HALO - (start - lo):] = x[b, lo:start + TOK]
        in_maps.append({
            "xh": np.ascontiguousarray(xh.astype(bf)),
            "wq": wq16, "wk": wk16, "wv": wv16, "wo": wo16,
            "bq": bq8, "bk": bk, "ones16": ones16, "msk2": m2s[c],
            "bob16": bob16, "onesr": onesr, "ident": ident,
        })
    _cache["last_in_maps"] = in_maps
    res = run_bass_kernel_spmd(nc, in_maps, list(range(8)))
    out = np.empty((B, S, D), np.float32)
    for core in range(8):
        b, c = divmod(core, 4)
        out[b, c * TOK:(c + 1) * TOK] = res.results[core]["out"]
    return out


# revision 37
# speedup vs baseline: 1.1886x; 1.1886x over previous
"""Local (causal, windowed) attention block on 8 trn2 NeuronCores.

Sharding: sequence-parallel. 8 shards = batch(2) x seq-quarter(4); each core
computes 512 output tokens and needs a 256-token K/V halo on the left.

All matmul operands are bf16 (psum accumulation stays f32): same 1 cycle/row
PE rate as fp32r but half the DMA/SBUF traffic and less PE power (fp32r at
full rate trips the DVFS utilization throttle). Data flows in transposed
[feature, token] layout so Q/K/scores/attn-out chain without transposes.
Softmax denominators come from an appended ones column in the V operand;
masking is multiplicative bf16 applied post-exp in SBUF (band masks doubled
per head pair), split across Pool and DVE. bv and bo fold host-side into
bo_eff = bv@wo + bo (softmax rows sum to 1), added via a ones-row matmul in
phase E. The attention loop is software-pipelined by one head pair; each
(pair, chunk) gets a two-bank psum tile (one accumulation group per bank --
sharing a bank hangs the HW) so a single strided exp covers both heads.
HW quirks found on the way: custom-DVE reciprocal must read SBUF at base
partition 0 (psum or base-64 reads give garbage), SB+SB tensor_tensor
operands must share a base partition, and stale SBUF from prior NEFF runs
must never reach a matmul (0 x garbage-NaN = NaN).
"""

import numpy as np
import ml_dtypes

import concourse.bass as bass  # noqa: F401
import concourse.mybir as mybir
import concourse.tile as tile
from concourse import bacc
from concourse.bass_utils import run_bass_kernel_spmd

B, S, D = 2, 2048, 1024
H, DH = 16, 64
WIN = 256
TOK, HALO = 512, 256
XT = TOK + HALO  # 768
F32 = mybir.dt.float32
F32R = mybir.dt.float32r
BF16 = mybir.dt.bfloat16

# per-key-chunk query windows (qlo, qwidth); kc=3 widened to 512 so the
# first attn@V matmul (start=True) covers the whole psum bank.
KC_WIN = [(0, 128), (0, 256), (0, 384), (0, 512), (256, 256), (384, 128)]
KC_ORDER = [3, 0, 1, 2, 4, 5]

_cache = {}


def r(ap):
    return ap.bitcast(F32R)


def build_nc():
    nc = bacc.Bacc(None, target_bir_lowering=False)
    xh_d = nc.declare_dram_parameter("xh", [XT, D], BF16, isOutput=False)
    msk2_d = nc.declare_dram_parameter("msk2", [6, 128, 1024], BF16, isOutput=False)
    wq_d = nc.declare_dram_parameter("wq", [D, D], BF16, isOutput=False)
    wk_d = nc.declare_dram_parameter("wk", [D, D], BF16, isOutput=False)
    wv_d = nc.declare_dram_parameter("wv", [D, D], BF16, isOutput=False)
    wo_d = nc.declare_dram_parameter("wo", [D, D], BF16, isOutput=False)
    bq_d = nc.declare_dram_parameter("bq", [D], F32, isOutput=False)  # pre-scaled 1/8
    bk_d = nc.declare_dram_parameter("bk", [D], F32, isOutput=False)
    ones16_d = nc.declare_dram_parameter("ones16", [128, 16], BF16, isOutput=False)
    onesr_d = nc.declare_dram_parameter("onesr", [1, 128], BF16, isOutput=False)
    bob16_d = nc.declare_dram_parameter("bob16", [1, D], BF16, isOutput=False)
    ident_d = nc.declare_dram_parameter("ident", [128, 128], BF16, isOutput=False)
    out_d = nc.declare_dram_parameter("out", [TOK, D], F32, isOutput=True)

    Exp = mybir.ActivationFunctionType.Exp
    Ident = mybir.ActivationFunctionType.Identity

    with tile.TileContext(nc) as tc:
        with (
            tc.tile_pool(name="const", bufs=1) as const,
            tc.tile_pool(name="persist", bufs=1) as persist,
            tc.tile_pool(name="w", bufs=9) as wpool,
        ):

            QT = [persist.tile([128, TOK], BF16, name=f"QT{i}") for i in range(8)]
            KT = [persist.tile([128, XT], BF16, name=f"KT{i}") for i in range(8)]
            Vg = [persist.tile([128, 16 * 65], BF16, name=f"Vg{i}") for i in range(6)]
            AO = [persist.tile([128, TOK], BF16, name=f"AO{i}") for i in range(8)]

            # ---- Phase A: load x (bf16), transpose to xT [feat, tok] ----
            ident = const.tile([128, 128], BF16)
            nc.sync.dma_start(out=ident[:], in_=ident_d[:, :])
            with tc.tile_pool(name="xTp", bufs=1) as xTp:
                xTt = [xTp.tile([128, XT], BF16, name=f"xT{i}") for i in range(8)]
                with (
                    tc.tile_pool(name="xload", bufs=3) as xpool,
                    tc.tile_pool(name="tpsum", bufs=4, space="PSUM") as tpsum,
                ):
                    for tt in range(6):
                        xt = xpool.tile([128, D], BF16, tag="xt")
                        nc.sync.dma_start(out=xt[:], in_=xh_d[tt * 128:(tt + 1) * 128, :])
                        for fc in range(8):
                            pt = tpsum.tile([128, 128], BF16, tag="tp")
                            nc.tensor.transpose(pt[:], xt[:, fc * 128:(fc + 1) * 128], ident[:])
                            if (tt * 8 + fc) % 2 == 0:
                                nc.vector.tensor_copy(xTt[fc][:, tt * 128:(tt + 1) * 128], pt[:])
                            else:
                                nc.scalar.copy(xTt[fc][:, tt * 128:(tt + 1) * 128], pt[:])
                wq_sb0 = []
                for kc in range(8):
                    wt = wpool.tile([128, D], BF16, tag="w", name=f"wq{kc}")
                    nc.sync.dma_start(out=wt[:], in_=wq_d[kc * 128:(kc + 1) * 128, :])
                    wq_sb0.append(wt)

                # ---- Phase B: QT = (wq.T @ xT)/8 + bq/8 ; KT = wk.T @ xT + bk ----
                wq_sb = wq_sb0
                # per-outcol-chunk bias columns: [:, 0:8]=bq/8, [:, 8:16]=bk
                bqk = const.tile([128, 16], F32)
                nc.sync.dma_start(out=bqk[:, 0:8], in_=bq_d[:].rearrange("(c p) -> p c", p=128))
                nc.sync.dma_start(out=bqk[:, 8:16], in_=bk_d[:].rearrange("(c p) -> p c", p=128))
                wk_sb = []
                for kc in range(8):
                    wt = wpool.tile([128, D], BF16, tag="w", name=f"wk{kc}")
                    nc.sync.dma_start(out=wt[:], in_=wk_d[kc * 128:(kc + 1) * 128, :])
                    wk_sb.append(wt)
                m2_sb = []
                for kc in range(6):
                    mt = const.tile([128, 1024], BF16, name=f"mask2_{kc}")
                    nc.sync.dma_start(out=mt[:], in_=msk2_d[kc])
                    m2_sb.append(mt)

                with tc.tile_pool(name="qpsum", bufs=4, space="PSUM") as qpsum:
                    for oc in range(8):
                        ps = qpsum.tile([128, TOK], F32, tag="pp")
                        for kc in range(8):
                            nc.tensor.matmul(
                                ps[:],
                                lhsT=wq_sb[kc][:, oc * 128:(oc + 1) * 128],
                                rhs=xTt[kc][:, HALO:XT],
                                start=(kc == 0), stop=(kc == 7),
                            )
                        # QT pre-scaled by 1/8 (bias arrives pre-scaled from host)
                        nc.scalar.activation(QT[oc][:], ps[:], Ident,
                                             bias=bqk[:, oc:oc + 1], scale=0.125)
                    for oc in range(8):
                        for hf in range(2):
                            ps = qpsum.tile([128, 384], F32, tag="pp", padded_shape=[128, 512])
                            for kc in range(8):
                                nc.tensor.matmul(
                                    ps[:],
                                    lhsT=wk_sb[kc][:, oc * 128:(oc + 1) * 128],
                                    rhs=xTt[kc][:, hf * 384:(hf + 1) * 384],
                                    start=(kc == 0), stop=(kc == 7),
                                )
                            nc.scalar.activation(KT[oc][:, hf * 384:(hf + 1) * 384],
                                                 ps[:], Ident,
                                                 bias=bqk[:, 8 + oc:9 + oc], scale=1.0)

                    # ---- Phase C: V (natural layout, no bias) + ones column ----
                    wv_sb = []
                    for kc in range(8):
                        wt = wpool.tile([128, D], BF16, tag="w", name=f"wv{kc}")
                        nc.sync.dma_start(out=wt[:], in_=wv_d[kc * 128:(kc + 1) * 128, :])
                        wv_sb.append(wt)
                    for tt in range(6):
                        for hf in range(2):
                            ps = qpsum.tile([128, 512], F32, tag="pp")
                            for kc in range(8):
                                nc.tensor.matmul(
                                    ps[:],
                                    lhsT=xTt[kc][:, tt * 128:(tt + 1) * 128],
                                    rhs=wv_sb[kc][:, hf * 512:(hf + 1) * 512],
                                    start=(kc == 0), stop=(kc == 7),
                                )
                            dst = Vg[tt][:, hf * 520:(hf + 1) * 520].rearrange(
                                "p (h d) -> p h d", d=65)[:, :, 0:64]
                            nc.scalar.copy(dst, ps[:].rearrange("p (h d) -> p h d", d=64))
                        nc.sync.dma_start(
                            out=Vg[tt][:].rearrange("p (h d) -> p h d", d=65)[:, :, 64:65],
                            in_=ones16_d[:, 0:16])

            # ---- Phase D: attention (head loop software-pipelined by 1) ----
            # prefetch wo while attention runs
            wo_sb = []
            for kc in range(8):
                wt = wpool.tile([128, D], BF16, tag="w", name=f"wo{kc}")
                nc.sync.dma_start(out=wt[:], in_=wo_d[kc * 128:(kc + 1) * 128, :])
                wo_sb.append(wt)
            onesr = wpool.tile([1, 128], BF16, name="onesr")
            nc.sync.dma_start(out=onesr[:], in_=onesr_d[:, :])
            bob16 = wpool.tile([1, D], BF16, name="bob16")
            nc.sync.dma_start(out=bob16[:], in_=bob16_d[:, :])

            with (
                tc.tile_pool(name="spsum", bufs=2, space="PSUM") as spsum,
                tc.tile_pool(name="opsum", bufs=2, space="PSUM") as opsum,
                tc.tile_pool(name="bpsum", bufs=2, space="PSUM") as bpsum,
                tc.tile_pool(name="es", bufs=17) as es_pool,
                tc.tile_pool(name="aou", bufs=3) as aou_pool,
                tc.tile_pool(name="den", bufs=2) as den_pool,
            ):
                es_all = [None] * 8

                def emit_scores_pair(p):
                    h0, h1 = 2 * p, 2 * p + 1
                    g = p
                    es_slices = {}
                    for kc in KC_ORDER:
                        qlo, qw = KC_WIN[kc]
                        # one two-bank psum tile per (pair, kc): each head's
                        # matmul owns a full bank (one accumulation group per
                        # bank — sharing a bank hangs the HW); a single exp
                        # reads both banks through a strided AP
                        ps = spsum.tile([128, 1024], F32, tag="sp")
                        for h2 in (0, 1):
                            ho = h2 * 64
                            nc.tensor.matmul(
                                ps[:, 512 * h2:512 * h2 + qw],
                                lhsT=KT[g][ho:ho + 64, kc * 128:(kc + 1) * 128],
                                rhs=QT[g][ho:ho + 64, qlo:qlo + qw],
                                start=True, stop=True,
                            )
                        es = es_pool.tile([128, 2 * qw], BF16, tag="es",
                                          padded_shape=[128, 1024])
                        nc.scalar.activation(
                            es[:].rearrange("p (b q) -> p b q", b=2),
                            ps[:].rearrange("p (b q) -> p b q", b=2)[:, :, 0:qw],
                            Exp)
                        with nc.allow_low_precision(reason="bf16 es mask"):
                            eng = nc.gpsimd if kc in (0, 1, 4, 5) else nc.vector
                            eng.tensor_mul(es[:], es[:], m2_sb[kc][:, 0:2 * qw])
                        es_slices[(0, kc)] = es[:, 0:qw]
                        es_slices[(1, kc)] = es[:, qw:2 * qw]
                    es_all[p] = es_slices

                def emit_av_pair(p):
                    g = p
                    es_slices = es_all[p]
                    aop = aou_pool.tile([128, TOK], F32, tag="ao")
                    den2 = den_pool.tile([1, 2 * TOK], F32, tag="dn")
                    for h2 in (0, 1):
                        h = 2 * p + h2
                        po = opsum.tile([65, TOK], F32, tag="op")
                        for i, kc in enumerate(KC_ORDER):
                            qlo, qw = KC_WIN[kc]
                            nc.tensor.matmul(
                                po[:, qlo:qlo + qw],
                                lhsT=Vg[kc][:, h * 65:(h + 1) * 65],
                                rhs=es_slices[(h2, kc)],
                                start=(i == 0), stop=(i == 5),
                            )
                        # evacuate psum immediately so the bank recycles fast;
                        # denominators pack into one base-0 row (the custom DVE
                        # reciprocal misreads non-zero base partitions on HW)
                        nc.vector.tensor_copy(aop[64 * h2:64 * h2 + 64, :], po[0:64, :])
                        nc.scalar.copy(den2[:, h2 * TOK:(h2 + 1) * TOK],
                                       po[64:65, :])
                    rsc2 = den_pool.tile([1, 2 * TOK], F32, tag="rs")
                    nc.vector.reciprocal_approx_fast(rsc2[:], den2[:])
                    rcb2 = den_pool.tile([1, 2 * TOK], BF16, tag="rb")
                    with nc.allow_low_precision(reason="bf16 1/den"):
                        nc.scalar.copy(rcb2[:], rsc2[:])
                    pb0 = bpsum.tile([64, TOK], F32, tag="bp")
                    nc.tensor.matmul(pb0[:], lhsT=onesr[:, 0:64],
                                     rhs=rcb2[:, 0:TOK], start=True, stop=True)
                    pb1 = bpsum.tile([64, TOK], F32, tag="bp")
                    nc.tensor.matmul(pb1[:], lhsT=onesr[:, 0:64],
                                     rhs=rcb2[:, TOK:2 * TOK], start=True, stop=True)
                    with nc.allow_low_precision(reason="bf16 attn output"):
                        nc.vector.tensor_mul(AO[g][0:64, :], pb0[:], aop[0:64, :])
                        nc.vector.tensor_mul(AO[g][64:128, :], pb1[:], aop[64:128, :])

                for p in range(9):
                    if p < 8:
                        emit_scores_pair(p)
                    if p >= 1:
                        emit_av_pair(p - 1)

            # ---- Phase E: out = AO.T @ wo + bo_eff ----
            with (
                tc.tile_pool(name="fpsum", bufs=5, space="PSUM") as fpsum,
                tc.tile_pool(name="oout", bufs=3) as oout,
            ):
                for tt in range(4):
                    ot = oout.tile([128, D], F32, tag="oo")
                    for hf in range(2):
                        ps = fpsum.tile([128, 512], F32, tag="fp")
                        nc.tensor.matmul(
                            ps[:], lhsT=onesr[:],
                            rhs=bob16[:, hf * 512:(hf + 1) * 512],
                            start=True, stop=False,
                        )
                        for kc in range(8):
                            nc.tensor.matmul(
                                ps[:],
                                lhsT=AO[kc][:, tt * 128:(tt + 1) * 128],
                                rhs=wo_sb[kc][:, hf * 512:(hf + 1) * 512],
                                start=False, stop=(kc == 7),
                            )
                        nc.scalar.copy(ot[:, hf * 512:(hf + 1) * 512], ps[:])
                        nc.sync.dma_start(
                            out=out_d[tt * 128:(tt + 1) * 128, hf * 512:(hf + 1) * 512],
                            in_=ot[:, hf * 512:(hf + 1) * 512])

    nc.compile()
    return nc


def _mask_for_chunk(c):
    m = np.zeros((6, 128, 512), np.float32)
    for kc in range(6):
        k = kc * 128 + np.arange(128)[:, None]
        q = np.arange(512)[None, :]
        valid = (q >= k - WIN) & (q <= k)
        if c == 0:
            valid = valid & (k >= HALO)
        m[kc][valid] = 1.0
    return m.astype(ml_dtypes.bfloat16)


def kernel(x, wq, bq, wk, bk, wv, bv, wo, bo):
    bf = ml_dtypes.bfloat16
    x = np.asarray(x, np.float32)
    wq16 = np.ascontiguousarray(np.asarray(wq, np.float32).astype(bf))
    wk16 = np.ascontiguousarray(np.asarray(wk, np.float32).astype(bf))
    wv16 = np.ascontiguousarray(np.asarray(wv, np.float32).astype(bf))
    wo32 = np.asarray(wo, np.float32)
    wo16 = np.ascontiguousarray(wo32.astype(bf))
    bq8 = np.ascontiguousarray(np.asarray(bq, np.float32) * 0.125)
    bk = np.ascontiguousarray(np.asarray(bk, np.float32))
    # fold bv through wo (softmax rows sum to 1): out += bv @ wo + bo
    bo_eff = (np.asarray(bv, np.float32) @ wo32 + np.asarray(bo, np.float32)).astype(np.float32)
    bob16 = np.ascontiguousarray(bo_eff.reshape(1, D).astype(bf))
    onesr = np.ones((1, 128), bf)
    ident = np.eye(128, dtype=np.float32).astype(bf)
    ones16 = np.ones((128, 16), bf)

    if "nc" not in _cache:
        _cache["nc"] = build_nc()
        _cache["masks"] = [_mask_for_chunk(c) for c in range(4)]
        m2s = []
        for c in range(4):
            m = _cache["masks"][c]
            m2 = np.zeros((6, 128, 1024), np.float32).astype(m.dtype)
            for kc in range(6):
                qlo, qw = KC_WIN[kc]
                sl = m[kc][:, qlo:qlo + qw]
                m2[kc][:, 0:qw] = sl
                m2[kc][:, qw:2 * qw] = sl
            m2s.append(m2)
        _cache["m2s"] = m2s
    nc = _cache["nc"]
    masks = _cache["masks"]
    m2s = _cache["m2s"]

    in_maps = []
    for core in range(8):
        b, c = divmod(core, 4)
        start = c * TOK
        xh = np.zeros((XT, D), np.float32)
        lo = max(0, start - HALO)
        xh[HALO - (start - lo):] = x[b, lo:start + TOK]
        in_maps.append({
            "xh": np.ascontiguousarray(xh.astype(bf)),
            "wq": wq16, "wk": wk16, "wv": wv16, "wo": wo16,
            "bq": bq8, "bk": bk, "ones16": ones16, "msk2": m2s[c],
            "bob16": bob16, "onesr": onesr, "ident": ident,
        })
    _cache["last_in_maps"] = in_maps
    res = run_bass_kernel_spmd(nc, in_maps, list(range(8)))
    out = np.empty((B, S, D), np.float32)
    for core in range(8):
        b, c = divmod(core, 4)
        out[b, c * TOK:(c + 1) * TOK] = res.results[core]["out"]
    return out
